# revision 1
# baseline (speedup 1.0000x reference)
"""Trainium2 Bass kernel for nn_BGATTNET_Loss (CE + pairwise cosine-sim regularizer).

Math
----
loss = CE(outputs, labels) + sum_b [ COE/n_pairs * sum_{i<j} cos(H[b,i], H[b,j]) ]

The O(N^2 D) pairwise term collapses to O(N D):
    sum_{i<j} cos_ij = 0.5 * ( || sum_n Hn_n ||^2  -  sum_n ||Hn_n||^2 )
with Hn_n = H_n / max(||H_n||, EPS).

Sharding: data-parallel over the bag dim B=8, one bag per NeuronCore.
Each core computes   partial_b = CE_b/8 + 0.5*COE/n_pairs * (ssq_b - N)
and the host sums the 8 scalars (rowssq is analytically N: unit-norm rows).

Per-core dataflow (bag H_b is [2048, 512] f32, streamed once = 4 MB,
which is the per-core HBM roofline ~12us at ~341 GB/s):
  - HWDGE DMA in tapered chunks (4,4,4,2,1,1 row-tiles of [128, 512]) so
    the stream saturates bandwidth while the post-last-byte tail is short
  - per-row sum-of-squares: ACT Square+accum_out on 6 tiles, DVE fused
    square+reduce (TensorScalarPtr w/ accum) on 10 (engine balance)
  - rnorm = 1/max(sqrt(sumsq), EPS) via int-magic + 1 Newton step on DVE
    only (no ACT table thrash; rel err <2e-3, irrelevant at reg's weight)
  - weighted column sum s = sum_n rnorm_n * H_n on the PE: per-tile
    matmul with the rnorm column as stationary operand, float32r (raw
    fp32, 1 cycle/row) accumulated in PSUM [1, 512]
  - ssq = CREG*||s||^2 read straight from PSUM by one ACT Square+accum
  - CE on-device: exp+accum -> ln on ACT (one table set pair, loaded in
    the first DMA's shadow), label select and combine on DVE
"""

from contextlib import ExitStack

import numpy as np

import concourse.bass as bass
import concourse.tile as tile
from concourse import bacc, mybir
from concourse._compat import axon_active
from concourse.bass_utils import run_bass_kernel_spmd

P = 128
B = 8
N = 2048
D = 512
NT = N // P  # 16 row tiles
G = 4  # row tiles per DMA group (1 MB)
NG = NT // G

COE = 0.01
N_PAIRS = N * (N - 1) / 2.0
CREG = float(0.5 * COE / N_PAIRS)
EPS = 1e-12

F32 = mybir.dt.float32
BF16 = mybir.dt.bfloat16
F32R = mybir.dt.float32r
I32 = mybir.dt.int32
AF = mybir.ActivationFunctionType
ALU = mybir.AluOpType

# Tiles whose sum-of-squares runs on ACT (Square+accum); the rest go to
# DVE (fused square+reduce). Balanced from the cost model (ACT ~799ns vs
# DVE ~594ns per tile); the final tile stays on DVE to keep the tail short.
ACT_SQ_TILES = frozenset({0, 3, 6, 9, 14, 15})

# int bit-trick seed for Newton rsqrt on DVE, pre-adjusted for a halved
# input: y0 = bits(MAGIC2 - (bits(x/2) >> 1)) approximates 1/sqrt(x)
RSQRT_MAGIC2 = 0x5F3759DF - 0x00400000


def _build_bass():
    nc = bacc.Bacc(
        "TRN2",
        target_bir_lowering=False,
        debug=not axon_active(),
        enable_asserts=False,
        num_devices=B,
    )

    # h is declared float32r (same bytes as f32): the PE's raw-fp32 matmul
    # mode needs f32r-typed producers end-to-end per the BIR verifier
    h = nc.dram_tensor("h", [N, D], F32R, kind="ExternalInput")
    xl_in = nc.dram_tensor("xl_in", [1, 3], F32, kind="ExternalInput")
    out = nc.dram_tensor("partial", [1, 1], F32, kind="ExternalOutput")

    hv = h[:, :].rearrange("(t p) d -> p t d", p=P)  # [128, 16, 512]

    with tile.TileContext(nc) as tc, ExitStack() as ctx:
        hpool = ctx.enter_context(tc.tile_pool(name="hbuf", bufs=6))
        scr_act = ctx.enter_context(tc.tile_pool(name="scr_act", bufs=2))
        scr_dve = ctx.enter_context(tc.tile_pool(name="scr_dve", bufs=2))
        grp = ctx.enter_context(tc.tile_pool(name="grp", bufs=2))
        stats = ctx.enter_context(tc.tile_pool(name="stats", bufs=1))
        small = ctx.enter_context(tc.tile_pool(name="small", bufs=1))
        psum = ctx.enter_context(tc.tile_pool(name="psum", bufs=1, space="PSUM"))

        sumsq = stats.tile([P, NT], F32)  # per-row ||H_n||^2
        rnorm = stats.tile([P, NT], F32R)  # per-row 1/max(||H_n||, EPS)
        magic = stats.tile([P, NT], I32)
        nc.vector.memset(magic, RSQRT_MAGIC2)

        s_acc = psum.tile([1, D], F32)  # sum_n rnorm_n * H_n

        # ---- CE for this core's bag (tiny; high priority so both ACT
        # table loads land in the early DMA shadow). lse computed without
        # max-shift (|outputs| ~ N(0,1), exp is safe in f32) so Exp and Ln
        # are adjacent ACT ops with no DVE round-trip between them. ----
        with tc.high_priority():
            x_sb = small.tile([1, 3], F32)
            nc.sync.dma_start(out=x_sb, in_=xl_in[:, :])

            e = small.tile([1, 2], F32)
            se = small.tile([1, 1], F32)
            nc.scalar.activation(e, x_sb[:, 0:2], AF.Exp, accum_out=se)
            lse = small.tile([1, 1], F32)
            lse_inst = nc.scalar.activation(lse, se, AF.Ln)
            dx = small.tile([1, 1], F32)
            nc.vector.tensor_tensor(dx, x_sb[:, 1:2], x_sb[:, 0:1], ALU.subtract)
            xl = small.tile([1, 1], F32)
            nc.vector.scalar_tensor_tensor(
                xl, in0=dx, scalar=x_sb[:, 2:3], in1=x_sb[:, 0:1],
                op0=ALU.mult, op1=ALU.add,
            )
            ce = small.tile([1, 1], F32)
            nc.vector.tensor_tensor(ce, lse, xl, ALU.subtract)
            ce8 = small.tile([1, 1], F32)
            nc.vector.tensor_scalar_mul(ce8, ce, 1.0 / B)
            # bias for the final ACT combine: ce/8 - CREG*N (rowssq = N)
            bias_pre = small.tile([1, 1], F32)
            nc.vector.tensor_scalar(
                bias_pre, in0=ce8, scalar1=float(N * CREG), scalar2=None,
                op0=ALU.subtract,
            )

        # ---- stream H: sumsq -> rnorm -> PE weighted column-sum ----
        def newton_rsqrt(ph):
            """rnorm[:, ph] = 1/sqrt(max(sumsq[:, ph], EPS^2)) on DVE only:
            quake int-magic seed + 1 Newton iteration (rel err <2e-3, well
            under the bf16 rounding the matmul already applies)."""
            w = ph.stop - ph.start
            xh = grp.tile([P, w], F32)
            nc.vector.tensor_scalar(
                xh, in0=sumsq[:, ph], scalar1=EPS * EPS, scalar2=0.5,
                op0=ALU.max, op1=ALU.mult,
            )
            yi = grp.tile([P, w], I32)
            nc.vector.tensor_scalar(
                yi, in0=xh[:, :].bitcast(I32), scalar1=1, scalar2=None,
                op0=ALU.arith_shift_right,
            )
            nc.vector.tensor_tensor(yi, magic[:, ph], yi, ALU.subtract)
            y = yi[:, :].bitcast(F32)
            a = grp.tile([P, w], F32)
            nc.vector.tensor_mul(a, y, y)
            nc.vector.tensor_mul(a, a, xh)
            nc.vector.tensor_scalar(
                a, in0=a, scalar1=-1.0, scalar2=1.5, op0=ALU.mult, op1=ALU.add
            )
            nc.vector.tensor_mul(rnorm[:, ph], y, a)  # f32 -> f32r out

        # tapered DMA chunks (big for bandwidth, small at the end) with
        # compute phases decoupled: each phase = rsqrt chain + matmuls over
        # tiles whose chunks have landed. The last two phases are narrow so
        # the after-last-byte tail is short.
        chunks = [(0, 4), (4, 8), (8, 12), (12, 14), (14, 15), (15, NT)]
        phases = [(0, 4), (4, 8), (8, 12), (12, 14), (14, NT)]
        hts = {}
        pidx = 0
        for lo, hi in chunks:
            ht = hpool.tile([P, hi - lo, D], F32R, tag="hbuf")
            nc.sync.dma_start(out=ht, in_=hv[:, lo:hi, :])

            for j in range(hi - lo):
                t = lo + j
                hts[t] = (ht, j)
                if t in ACT_SQ_TILES:
                    # ACT path: Square with free-dim accumulate
                    sa = scr_act.tile([P, D], F32)
                    sq_inst = nc.scalar.activation(
                        sa, ht[:, j, :].bitcast(F32), AF.Square,
                        accum_out=sumsq[:, t : t + 1],
                    )
                    if t == 0:
                        # order-only edge: CE's Ln (and its table load) must
                        # precede the first square so the natural_log table
                        # load lands in the early DMA shadow, not mid-stream
                        tile.add_dep_helper(
                            sq_inst.ins, lse_inst.ins, sync=False,
                            reason="ACT table load before square stream",
                        )
                else:
                    # DVE path: fused square+reduce (TensorScalarPtr w/ accum)
                    sv = scr_dve.tile([P, D], F32)
                    nc.vector.scalar_tensor_tensor(
                        sv, in0=ht[:, j, :].bitcast(F32), scalar=1.0,
                        in1=ht[:, j, :].bitcast(F32),
                        op0=ALU.mult, op1=ALU.mult,
                        accum_out=sumsq[:, t : t + 1],
                    )

            while pidx < len(phases) and phases[pidx][1] <= hi:
                plo, phi = phases[pidx]
                pidx += 1
                newton_rsqrt(slice(plo, phi))
                for t in range(plo, phi):
                    tht, j = hts[t]
                    nc.tensor.matmul(
                        s_acc[:, :],
                        lhsT=rnorm[:, t : t + 1],
                        rhs=tht[:, j, :],
                        start=(t == 0),
                        stop=(t == NT - 1),
                    )

        # ---- finals, all on ACT so no cross-engine hop before the output:
        # ssq = CREG*||s||^2 straight from PSUM (Square of sqrt(CREG)*s with
        # accumulate), then partial = ssq + (ce/8 - CREG*N) via Identity
        sq_s = psum.tile([1, D], F32)
        ssq = small.tile([1, 1], F32)
        nc.scalar.activation(
            sq_s, s_acc, AF.Square, scale=float(np.sqrt(CREG)), accum_out=ssq
        )
        part = small.tile([1, 1], F32)
        nc.scalar.activation(part, ssq, AF.Identity, bias=bias_pre[:, :])
        nc.scalar.dma_start(out=out[:, :], in_=part)

    nc.compile()
    return nc


_NC_CACHE = None


def _get_nc():
    global _NC_CACHE
    if _NC_CACHE is None:
        _NC_CACHE = _build_bass()
    return _NC_CACHE


def _run(inputs, trace=False, **kwargs):
    outputs = np.asarray(inputs["outputs"], dtype=np.float32)
    labels = np.asarray(inputs["labels"])
    H = np.asarray(inputs["H"], dtype=np.float32)
    assert H.shape == (B, N, D), H.shape

    in_maps = []
    for b in range(B):
        in_maps.append(
            {
                "h": np.ascontiguousarray(H[b]),
                "xl_in": np.array(
                    [[outputs[b, 0], outputs[b, 1], float(labels[b])]],
                    dtype=np.float32,
                ),
            }
        )
    res = run_bass_kernel_spmd(
        _get_nc(), in_maps, core_ids=list(range(B)), trace=trace, **kwargs
    )
    partials = [float(r["partial"][0, 0]) for r in res.results]
    total = np.float32(sum(partials))
    return np.asarray(total, dtype=np.float32), res


def kernel(**inputs) -> np.ndarray:
    total, _ = _run(inputs, trace=False)
    return total



# revision 3
# speedup vs baseline: 1.3112x; 1.3112x over previous
"""Trainium2 Bass kernel for nn_BGATTNET_Loss (CE + pairwise cosine-sim regularizer).

Math
----
loss = CE(outputs, labels) + sum_b [ COE/n_pairs * sum_{i<j} cos(H[b,i], H[b,j]) ]

The O(N^2 D) pairwise term collapses to O(N D):
    sum_{i<j} cos_ij = 0.5 * ( || sum_n Hn_n ||^2  -  N )
with Hn_n = H_n / ||H_n|| (unit rows, so sum_n ||Hn_n||^2 = N analytically).

Sharding: data-parallel over the bag dim B=8, one bag per NeuronCore.
Each core computes   partial_b = CE_b/8 + CREG * (ssq_b - N)
and the host sums the 8 scalars.

Per-core dataflow (bag H_b is [2048, 512] f32 in HBM):
  - SWDGE cast-DMA (gpsimd) streams H f32 -> bf16 SBUF in 5 chunks; the
    Pool engine generates descriptors one chunk ahead so the DMA engines
    stay saturated end to end. bf16 costs half the DMA time of f32 and is
    far more precision than the regularizer needs (it contributes ~1e-6 of
    the loss; tolerance is 2e-2).
  - per-row sum-of-squares split DVE (fused mult+reduce custom op) / ACT
    (Square with free-dim accumulate) for engine balance
  - rnorm = 1/sqrt(sumsq) via quake-seed + 1 Newton step on DVE, output
    bf16 (rel err <2e-3, irrelevant at the reg term's weight)
  - s = sum_n rnorm_n * H_n on the PE as 4 accumulation chains: per tile,
    4 matmuls with the H 128x128 d-block as the *stationary* operand and
    the rnorm column [128,1] moving -> PSUM s_acc[128, 4] (d on partitions)
  - ssq: DVE squares s_acc (scaled by CREG) -> SBUF bf16, PE ones-matmul
    reduces partitions -> G[1,4], ACT Identity(+bias)+accumulate folds in
    the CE partial, ACT DMAs the scalar out
  - CE on-device: exp+accum -> ln on ACT early (both table loads land in
    the DMA shadow), label select and combine on DVE
"""

from contextlib import ExitStack

import numpy as np

import concourse.bass as bass
import concourse.tile as tile
from concourse import bacc, mybir
from concourse._compat import axon_active
from concourse.bass_utils import run_bass_kernel_spmd
from concourse.dve_ops import TENSOR_TENSOR_REDUCE

P = 128
B = 8
N = 2048
D = 512
NT = N // P  # 16 row tiles
NDB = D // P  # 4 dim blocks

COE = 0.01
N_PAIRS = N * (N - 1) / 2.0
CREG = float(0.5 * COE / N_PAIRS)

F32 = mybir.dt.float32
BF16 = mybir.dt.bfloat16
I32 = mybir.dt.int32
AF = mybir.ActivationFunctionType
ALU = mybir.AluOpType

# DMA chunks (in 128-row tiles). First chunk is sized so its descriptor
# generation finishes before chunk 2's, keeping the SDMA stream gapless;
# the last is a single tile so the tail dependency chain is short.
CHUNKS = [(0, 3), (3, 7), (7, 11), (11, 15), (15, 16)]

# Tiles whose sum-of-squares runs on ACT (Square+accum, 799ns each); the
# rest go to the DVE fused mult+reduce (594ns each). Alternating within
# each chunk keeps both engines fed at chunk cadence.
ACT_SQ_TILES = frozenset({1, 4, 6, 8, 10, 12, 14})

RSQRT_MAGIC = 0x5F3759DF


def _build_bass():
    nc = bacc.Bacc(
        "TRN2",
        target_bir_lowering=False,
        debug=not axon_active(),
        enable_asserts=False,
        num_devices=B,
    )

    h = nc.dram_tensor("h", [N, D], F32, kind="ExternalInput")
    xl_in = nc.dram_tensor("xl_in", [1, 3], F32, kind="ExternalInput")
    out = nc.dram_tensor("partial", [1, 1], F32, kind="ExternalOutput")

    hv = h[:, :].rearrange("(t p) d -> p t d", p=P)  # [128, 16, 512]

    with tile.TileContext(nc) as tc, ExitStack() as ctx:
        hpool = ctx.enter_context(tc.tile_pool(name="hbuf", bufs=len(CHUNKS)))
        scr_act = ctx.enter_context(tc.tile_pool(name="scr_act", bufs=2))
        scr_dve = ctx.enter_context(tc.tile_pool(name="scr_dve", bufs=2))
        grp = ctx.enter_context(tc.tile_pool(name="grp", bufs=2))
        stats = ctx.enter_context(tc.tile_pool(name="stats", bufs=1))
        small = ctx.enter_context(tc.tile_pool(name="small", bufs=1))
        psum = ctx.enter_context(tc.tile_pool(name="psum", bufs=1, space="PSUM"))

        sumsq = stats.tile([P, NT], F32)  # per-row ||H_n||^2
        rnorm = stats.tile([P, NT], BF16)  # per-row 1/||H_n||
        magic = stats.tile([P, NT], I32)
        nc.vector.memset(magic, RSQRT_MAGIC)
        ones = stats.tile([P, 1], BF16)
        nc.vector.memset(ones, 1.0)

        s_acc = psum.tile([P, NDB], F32)  # s = sum_n rnorm_n*H_n, d on partitions
        gacc = psum.tile([1, NDB], F32)

        # ---- CE for this core's bag (tiny; high priority so the ACT table
        # loads land in the early DMA shadow). lse computed without
        # max-shift (|outputs| ~ N(0,1), exp is safe in f32) so Exp and Ln
        # are adjacent ACT ops. ----
        with tc.high_priority():
            x_sb = small.tile([1, 3], F32)
            nc.sync.dma_start(out=x_sb, in_=xl_in[:, :])

            e = small.tile([1, 2], F32)
            se = small.tile([1, 1], F32)
            nc.scalar.activation(e, x_sb[:, 0:2], AF.Exp, accum_out=se)
            lse = small.tile([1, 1], F32)
            lse_inst = nc.scalar.activation(lse, se, AF.Ln)
            dx = small.tile([1, 1], F32)
            nc.vector.tensor_tensor(dx, x_sb[:, 1:2], x_sb[:, 0:1], ALU.subtract)
            xl = small.tile([1, 1], F32)
            nc.vector.scalar_tensor_tensor(
                xl, in0=dx, scalar=x_sb[:, 2:3], in1=x_sb[:, 0:1],
                op0=ALU.mult, op1=ALU.add,
            )
            ce = small.tile([1, 1], F32)
            nc.vector.tensor_tensor(ce, lse, xl, ALU.subtract)
            # bias for the final ACT accumulate over G[1,4]:
            # partial = sum_j (G_j + bias4) = CREG*ssq + ce/8 - CREG*N
            # so bias4 = ce/(8*4) - CREG*N/4
            bias4 = small.tile([1, 1], F32)
            nc.vector.tensor_scalar(
                bias4, in0=ce, scalar1=1.0 / (8 * NDB),
                scalar2=float(N * CREG / NDB),
                op0=ALU.mult, op1=ALU.subtract,
            )

        # ---- stream H (f32 -> bf16 cast in DMA): sumsq -> rnorm -> PE ----
        def newton_rsqrt(lo, hi):
            """rnorm[:, lo:hi] = 1/sqrt(sumsq[:, lo:hi]) on DVE: quake
            int-magic seed + 1 Newton iteration (rel err <2e-3)."""
            ph = slice(lo, hi)
            w = hi - lo
            yi = grp.tile([P, w], I32)
            nc.vector.tensor_scalar(
                yi, in0=sumsq[:, ph].bitcast(I32), scalar1=1, scalar2=None,
                op0=ALU.arith_shift_right,
            )
            nc.vector.tensor_tensor(yi, magic[:, ph], yi, ALU.subtract)
            y = yi[:, :].bitcast(F32)
            a = grp.tile([P, w], F32)
            nc.vector.tensor_mul(a, y, y)
            nc.vector.tensor_mul(a, a, sumsq[:, ph])
            nc.vector.tensor_scalar(
                a, in0=a, scalar1=-0.5, scalar2=1.5, op0=ALU.mult, op1=ALU.add
            )
            nc.vector.tensor_mul(rnorm[:, ph], y, a)  # f32 -> bf16 out

        first_sq_inst = None
        for lo, hi in CHUNKS:
            w = hi - lo
            ht = hpool.tile([P, w, D], BF16, tag="hbuf")
            nc.gpsimd.dma_start(out=ht, in_=hv[:, lo:hi, :])

            for j in range(w):
                t = lo + j
                if t in ACT_SQ_TILES:
                    sa = scr_act.tile([P, D], BF16)
                    sq_inst = nc.scalar.activation(
                        sa, ht[:, j, :], AF.Square,
                        accum_out=sumsq[:, t : t + 1],
                    )
                    if first_sq_inst is None:
                        first_sq_inst = sq_inst
                else:
                    sv = scr_dve.tile([P, D], BF16)
                    nc.vector._custom_dve(
                        TENSOR_TENSOR_REDUCE,
                        out=sv, in0=ht[:, j, :], in1=ht[:, j, :],
                        s0=0.0, s1=1.0,
                        accum_out=sumsq[:, t : t + 1],
                    )

            newton_rsqrt(lo, hi)
            for j in range(w):
                t = lo + j
                for db in range(NDB):
                    nc.tensor.matmul(
                        s_acc[:, db : db + 1],
                        lhsT=ht[:, j, db * P : (db + 1) * P],
                        rhs=rnorm[:, t : t + 1],
                        start=(t == 0),
                        stop=(t == NT - 1),
                    )

        # order-only edge: CE's Ln (and the ACT table loads) must precede
        # the first ACT square so the loads land in the early DMA shadow
        if first_sq_inst is not None:
            tile.add_dep_helper(
                first_sq_inst.ins, lse_inst.ins, sync=False,
                reason="ACT table loads before square stream",
            )

        # ---- finals: sq = CREG*s*s (DVE, PSUM->SBUF bf16), partition-sum
        # via PE ones-matmul -> G[1,4], then one ACT Identity w/ bias +
        # accumulate folds in CE and emits the scalar; ACT DMAs it out.
        sq_sb = small.tile([P, NDB], BF16)
        nc.scalar.activation(
            sq_sb, s_acc, AF.Square, scale=float(np.sqrt(CREG))
        )
        nc.tensor.matmul(gacc, lhsT=ones, rhs=sq_sb, start=True, stop=True)
        gid = small.tile([1, NDB], F32)
        part = small.tile([1, 1], F32)
        nc.scalar.activation(
            gid, gacc, AF.Identity, bias=bias4[:, :], accum_out=part
        )
        nc.scalar.dma_start(out=out[:, :], in_=part)

    nc.compile()
    return nc


_NC_CACHE = None


def _get_nc():
    global _NC_CACHE
    if _NC_CACHE is None:
        _NC_CACHE = _build_bass()
    return _NC_CACHE


def _run(inputs, trace=False, **kwargs):
    outputs = np.asarray(inputs["outputs"], dtype=np.float32)
    labels = np.asarray(inputs["labels"])
    H = np.asarray(inputs["H"], dtype=np.float32)
    assert H.shape == (B, N, D), H.shape

    in_maps = []
    for b in range(B):
        in_maps.append(
            {
                "h": np.ascontiguousarray(H[b]),
                "xl_in": np.array(
                    [[outputs[b, 0], outputs[b, 1], float(labels[b])]],
                    dtype=np.float32,
                ),
            }
        )
    res = run_bass_kernel_spmd(
        _get_nc(), in_maps, core_ids=list(range(B)), trace=trace, **kwargs
    )
    partials = [float(r["partial"][0, 0]) for r in res.results]
    total = np.float32(sum(partials))
    return np.asarray(total, dtype=np.float32), res


def kernel(**inputs) -> np.ndarray:
    total, _ = _run(inputs, trace=False)
    return total
